# revision 1
# baseline (speedup 1.0000x reference)
"""Causal self-attention (GQA + RoPE) Trainium2 kernel, 8-way sharded.

Sharding: DP=4 over batch x TP=2 over kv-head groups (2 kv heads + their
8 q heads per group).  Each core computes its batch's qkv projection for
its head group, causal attention, and a partial c_proj (columns of
w_proj for its head group).  Host sums the two partial c_proj outputs
per batch.

Everything on-chip runs transposed ([feature, token] layout) so matmuls
contract along partitions; host transposes inputs/outputs.

Pipeline: the attention inner loop is ACT-bound (one exp per QK tile),
so the q/k projection + RoPE work for head h+1 is interleaved into the
PE stream of head h's attention, keeping the PE busy while ACT churns
through exps.

RoPE: w_attn q/k rows are permuted per-head to [even dims; odd dims] so
rotation pairs land at partition f and f+64 of the qkv psum tile:
  P  = ps * [c; c] (SBUF),  P2 = ps * [s; s] (PSUM)
  out[0:64]   = P[0:64]  - P2[64:128]
  out[64:128] = P2[0:64] + P[64:128]
(each combine reads one SBUF + one PSUM operand, which may sit at
different base partitions; two SBUF operands may not).

Softmax: att^T tiles ([k, q] layout) are exp'd on ACT without
max-subtraction (logits are O(6), fp32-safe).  Denominators: groups of
4 e-tiles are tree-summed on DVE and hit with one ones-column matmul
per group (deferred into the next group's PE stream); the per-q
reciprocal is broadcast down partitions with a f32r outer-product
matmul, also deferred one q-tile.
"""

import math

import numpy as np
import ml_dtypes

import concourse.bass as bass
import concourse.mybir as mybir
import concourse.tile as tile
from concourse import bacc
from concourse.bass_utils import run_bass_kernel_spmd

ALU = mybir.AluOpType
AF = mybir.ActivationFunctionType
F32 = mybir.dt.float32
F32R = mybir.dt.float32r
BF16 = mybir.dt.bfloat16
BF = ml_dtypes.bfloat16

# problem shape (hardcoded per contest rules)
B, T, C = 4, 2048, 2048
N_HEAD, N_KV_HEAD, HD = 16, 4, 128
ROPE_THETA = 10000.0

TP = 2            # head-group shards
DP = 4            # batch shards
HQ = N_HEAD // TP         # 8 q heads per core
HKV = N_KV_HEAD // TP     # 2 kv heads per core
NREP = N_HEAD // N_KV_HEAD  # 4
QK_ROWS = (HQ + HKV) * HD   # 1280
KC = C // 128     # 16 contraction tiles
NQ = T // 512     # 4 token strips
MQK = QK_ROWS // 128  # 10 feature tiles (8 q heads + 2 kv heads)
FM = C // 128     # 16 output feature tiles
SCALE = 1.0 / math.sqrt(HD)

N_CORES = 8

_NC = None        # cached compiled Bass module
LAST_RUN = None   # BassKernelResults of the most recent kernel() call


def build_nc(dbg=False):
    nc = bacc.Bacc(None, target_bir_lowering=False, debug=False)

    xT = nc.declare_dram_parameter("xT", [C, T], BF16, isOutput=False)
    wqk3 = nc.declare_dram_parameter("wqk3", [MQK, 128, C], BF16, isOutput=False)
    wv3 = nc.declare_dram_parameter("wv3", [128, KC * HKV * HD], BF16, isOutput=False)
    wp5 = nc.declare_dram_parameter("wp5", [FM, 128, HQ, 128], BF16, isOutput=False)
    trigf = nc.declare_dram_parameter("trigf", [128, T], F32, isOutput=False)  # [c;c]
    trigw = nc.declare_dram_parameter("trigw", [128, T], F32, isOutput=False)  # [s;s]
    maskd = nc.declare_dram_parameter("maskd", [4, 128, 512], BF16, isOutput=False)
    outT = nc.declare_dram_parameter("outT", [C, T], F32, isOutput=True)
    if dbg:
        dbg_q = nc.declare_dram_parameter("dbg_q", [128, T], BF16, isOutput=True)
        dbg_k = nc.declare_dram_parameter("dbg_k", [128, T], BF16, isOutput=True)
        dbg_v = nc.declare_dram_parameter(
            "dbg_v", [128, T // 128, HKV * HD], BF16, isOutput=True
        )
        dbg_y = nc.declare_dram_parameter("dbg_y", [128, HQ, T], BF16, isOutput=True)

    with tile.TileContext(nc) as tc:
        with (
            tc.tile_pool(name="const", bufs=1) as const,
            tc.tile_pool(name="persist", bufs=1) as persist,
            tc.tile_pool(name="eb", bufs=6) as eb,
            tc.tile_pool(name="gag", bufs=2) as gag,
            tc.tile_pool(name="rb", bufs=1) as rb,
            tc.tile_pool(name="psS", bufs=2, space="PSUM") as psS,
            tc.tile_pool(name="psY", bufs=2, space="PSUM") as psY,
            tc.tile_pool(name="psD", bufs=2, space="PSUM") as psD,
        ):
            trigf_sb = const.tile([128, T], F32, name="trigf")
            trigw_sb = const.tile([128, T], F32, name="trigw")
            mask_sb = const.tile([128, 4, 512], BF16, name="mask")
            ones_col = const.tile([128, 1], BF16, name="onec")
            ones_row_f = const.tile([1, 128], F32, name="onerf")
            ones_row = const.tile([1, 128], F32R, name="oner")

            qrot = [persist.tile([128, T], BF16, name=f"qrot{h}") for h in range(HQ)]
            krot = [persist.tile([128, T], BF16, name=f"krot{h}") for h in range(HKV)]
            v_sb = persist.tile([128, T // 128, HKV * HD], BF16, name="vtok")
            yt = persist.tile([128, HQ, T], BF16, name="yt")

            state = {"pending": None, "pending_ones": None}

            def finalize(h, qj, ps_y, ps_d):
                rec_f = rb.tile([1, 512], F32, name="recf")
                rec_r = rb.tile([1, 512], F32R, name="recr")
                r_sb = rb.tile([128, 512], F32, name="r")
                nc.vector.reciprocal(rec_f[:], ps_d[:])
                with nc.allow_low_precision("f32r recip broadcast"):
                    nc.vector.tensor_copy(rec_r[:], rec_f[:])
                ps_r = psS.tile([128, 512], F32, name="pss")
                nc.tensor.matmul(
                    ps_r[:], ones_row[:], rec_r[:], start=True, stop=True
                )
                nc.scalar.copy(r_sb[:], ps_r[:])
                nc.vector.tensor_tensor(
                    yt[:, h, bass.ts(qj, 512)], ps_y[:], r_sb[:], ALU.mult
                )

            def flush_ones():
                if state["pending_ones"] is not None:
                    po, st, sp, pd = state["pending_ones"]
                    nc.tensor.matmul(pd[:], ones_col[:], po[:], start=st, stop=sp)
                    state["pending_ones"] = None

            def emit_qj(h, qj, pop):
                """Attention for (h, qj): QK tiles, exp, mask, AV, denominators.

                `pop()` is called once per k-tile to interleave filler PE work.
                Returns the (ps_y, ps_d) accumulators (not yet finalized).
                """
                kvh = h // NREP
                qsl = bass.ts(qj, 512)
                ps_y = psY.tile([128, 512], F32, name="psy")
                ps_d = psD.tile([1, 512], F32, name="psd")
                nkt = 4 * qj + 4
                g0 = ga = g2 = None
                for kt in range(nkt):
                    d = kt - 4 * qj
                    # diagonal tile d has valid q-columns only in [128d, 512)
                    lo = 128 * d if d > 0 else 0
                    qlo = qj * 512 + lo
                    ps_s = psS.tile([128, 512], F32, name="pss")
                    nc.tensor.matmul(
                        ps_s[:, lo:512],
                        krot[kvh][:, kt * 128 : (kt + 1) * 128],
                        qrot[h][:, qlo : (qj + 1) * 512],
                        start=True,
                        stop=True,
                    )
                    e = eb.tile([128, 512], BF16, name="e")
                    nc.scalar.activation(
                        e[:, lo:512], ps_s[:, lo:512], AF.Exp, scale=SCALE
                    )
                    if d >= 0:
                        nc.vector.tensor_tensor(
                            e[:, lo:512], e[:, lo:512],
                            mask_sb[:, d, lo:512], ALU.mult,
                        )
                    nc.tensor.matmul(
                        ps_y[:, lo:512],
                        v_sb[:, kt, kvh * HD : (kvh + 1) * HD],
                        e[:, lo:512],
                        start=(kt == 0),
                        stop=(kt == nkt - 1),
                    )
                    if d >= 0:
                        # diagonal group: narrow per-tile ones-matmuls
                        if d == 0:
                            flush_ones()
                        nc.tensor.matmul(
                            ps_d[:, lo:512],
                            ones_col[:],
                            e[:, lo:512],
                            start=(qj == 0 and kt == 0),
                            stop=(kt == nkt - 1),
                        )
                    else:
                        # full groups: tree-sum 4 e-tiles on DVE, one deferred
                        # ones-matmul per group (emitted in a later PE slot so
                        # the PE never waits on the DVE adds).
                        ph = kt % 4
                        if ph == 0:
                            g0 = e
                        elif ph == 1:
                            ga = gag.tile([128, 512], BF16, name="ga")
                            nc.vector.tensor_tensor(ga[:], g0[:], e[:], ALU.add)
                        elif ph == 2:
                            g2 = e
                        else:
                            gs = gag.tile([128, 512], BF16, name="gs")
                            nc.vector.tensor_tensor(gs[:], g2[:], e[:], ALU.add)
                            nc.vector.tensor_tensor(gs[:], gs[:], ga[:], ALU.add)
                            flush_ones()
                            grp = kt // 4
                            state["pending_ones"] = (gs, grp == 0, False, ps_d)
                    pop(kt)
                return ps_y, ps_d

            # ======== projection machinery (lives through heads 0..6) ========
            with (
                tc.tile_pool(name="xa", bufs=1) as xa,
                tc.tile_pool(name="wm", bufs=3) as wm,
                tc.tile_pool(name="ta", bufs=1) as ta,
                tc.tile_pool(name="psA", bufs=1, space="PSUM") as psA,
                tc.tile_pool(name="psP2", bufs=1, space="PSUM") as psP2,
            ):
                xs = xa.tile([128, KC, T], BF16, name="xs")

                def load_wm(m):
                    w = wm.tile([128, KC, 128], BF16, name="wm")
                    wsrc = wqk3[m, :, :].rearrange("p (kc c) -> p kc c", kc=KC)
                    for i in range(4):
                        nc.sync.dma_start(
                            w[:, 4 * i : 4 * i + 4, :], wsrc[:, 4 * i : 4 * i + 4, :]
                        )
                    return w

                def rope_ops(m, n, ps):
                    """The four RoPE ops for one (feature tile, strip) pair."""
                    dst = qrot[m] if m < HQ else krot[m - HQ]
                    nsl = bass.ts(n, 512)
                    pt = ta.tile([128, 512], F32, name="pt")
                    p2 = psP2.tile([128, 512], F32, name="p2")
                    yield nc.vector.tensor_tensor(
                        pt[:], ps[:], trigf_sb[:, nsl], ALU.mult
                    )
                    yield nc.vector.tensor_tensor(
                        p2[:], ps[:], trigw_sb[:, nsl], ALU.mult
                    )
                    yield nc.vector.tensor_tensor(
                        dst[0:64, nsl], pt[0:64, :], p2[64:128, :], ALU.subtract
                    )
                    yield nc.vector.tensor_tensor(
                        dst[64:128, nsl], p2[0:64, :], pt[64:128, :], ALU.add
                    )

                def a_stream(m, pool):
                    w = load_wm(m)
                    yield
                    for n in range(NQ):
                        nsl = bass.ts(n, 512)
                        ps = pool.tile([128, 512], F32, name="psA")
                        for kc in range(KC):
                            nc.tensor.matmul(
                                ps[:],
                                w[:, kc, :],
                                xs[:, kc, nsl],
                                start=(kc == 0),
                                stop=(kc == KC - 1),
                            )
                            if kc % 2 == 1:
                                yield
                        for _ in rope_ops(m, n, ps):
                            yield

                # ---- A0: v projection + k heads + q head 0 (pure PE phase) ----
                with tc.tile_pool(name="wvp", bufs=1) as wvp:
                    wv_sb = wvp.tile([128, KC, HKV * HD], BF16, name="wv")
                    wvsrc = wv3.rearrange("p (kc c) -> p kc c", kc=KC)
                    for i in range(4):
                        nc.sync.dma_start(
                            wv_sb[:, 4 * i : 4 * i + 4, :],
                            wvsrc[:, 4 * i : 4 * i + 4, :],
                        )
                    for kc in range(KC):
                        nc.sync.dma_start(
                            xs[:, kc, bass.ts(0, 512)],
                            xT[kc * 128 : (kc + 1) * 128, bass.ts(0, 512)],
                        )
                    nc.sync.dma_start(trigf_sb[:], trigf[:])
                    nc.sync.dma_start(trigw_sb[:], trigw[:])
                    nc.sync.dma_start(mask_sb[:], maskd.rearrange("d p q -> p d q"))
                    nc.vector.memset(ones_col[:], 1.0)
                    nc.vector.memset(ones_row_f[:], 1.0)
                    with nc.allow_low_precision("f32r ones for recip broadcast"):
                        nc.vector.tensor_copy(ones_row[:], ones_row_f[:])
                    wk0 = load_wm(HQ)
                    wk1 = load_wm(HQ + 1)
                    wq0 = load_wm(0)
                    for n in range(NQ):
                        nsl = bass.ts(n, 512)
                        if n + 1 < NQ:
                            nsl_next = bass.ts(n + 1, 512)
                            for kc in range(KC):
                                nc.sync.dma_start(
                                    xs[:, kc, nsl_next],
                                    xT[kc * 128 : (kc + 1) * 128, nsl_next],
                                )
                        for tt in range(4 * n, 4 * n + 4):
                            # reuse the attention-phase psum slots during A0
                            psv = psS.tile([128, 512], F32, name="pss")[
                                :, : HKV * HD
                            ]
                            for kc in range(KC):
                                nc.tensor.matmul(
                                    psv[:],
                                    xs[:, kc, tt * 128 : (tt + 1) * 128],
                                    wv_sb[:, kc, :],
                                    start=(kc == 0),
                                    stop=(kc == KC - 1),
                                )
                            nc.scalar.copy(v_sb[:, tt, :], psv[:])
                        for m, w in ((HQ, wk0), (HQ + 1, wk1), (0, wq0)):
                            ps = psY.tile([128, 512], F32, name="psy")
                            for kc in range(KC):
                                nc.tensor.matmul(
                                    ps[:],
                                    w[:, kc, :],
                                    xs[:, kc, nsl],
                                    start=(kc == 0),
                                    stop=(kc == KC - 1),
                                )
                            for _ in rope_ops(m, n, ps):
                                pass

                # ---- heads 0..6: attention + next head's projection ----
                for h in range(HQ - 1):
                    agen = a_stream(h + 1, psA)

                    def pop(kt, agen=agen):
                        next(agen, None)
                        if kt < 5 or kt >= 10:
                            next(agen, None)

                    for qj in range(NQ):
                        ps_y, ps_d = emit_qj(h, qj, pop)
                        if state["pending"] is not None:
                            finalize(*state["pending"])
                        state["pending"] = (h, qj, ps_y, ps_d)
                    for _ in agen:
                        pass

            # ---- head 7: attention + output projection interleaved ----
            with (
                tc.tile_pool(name="wpc", bufs=3) as wpc,
                tc.tile_pool(name="obp", bufs=3) as obp,
                tc.tile_pool(name="psO", bufs=2, space="PSUM") as psO,
            ):
                def c_stream(n):
                    """Output projection for token strip n (16 feature tiles)."""
                    nsl = bass.ts(n, 512)
                    for fm in range(FM):
                        wmc = wpc.tile([128, HQ, 128], BF16, name="wpc")
                        nc.sync.dma_start(wmc[:], wp5[fm, :, :, :])
                        yield
                        ps_o = psO.tile([128, 512], F32, name="pso")
                        for h2 in range(HQ):
                            nc.tensor.matmul(
                                ps_o[:],
                                wmc[:, h2, :],
                                yt[:, h2, nsl],
                                start=(h2 == 0),
                                stop=(h2 == HQ - 1),
                            )
                            if h2 % 2 == 1:
                                yield
                        ob = obp.tile([128, 512], F32, name="ob")
                        nc.scalar.copy(ob[:], ps_o[:])
                        nc.sync.dma_start(
                            outT[fm * 128 : (fm + 1) * 128, nsl], ob[:]
                        )
                        yield

                cgens = []

                _end = object()

                def pop7(kt):
                    for _ in range(2):
                        while cgens:
                            if next(cgens[0], _end) is _end:
                                cgens.pop(0)
                                continue
                            break

                for qj in range(NQ):
                    ps_y, ps_d = emit_qj(HQ - 1, qj, pop7)
                    flush_ones()
                    if state["pending"] is not None:
                        finalize(*state["pending"])
                        state["pending"] = None
                    finalize(HQ - 1, qj, ps_y, ps_d)
                    cgens.append(c_stream(qj))
                # drain remaining output projection
                for g in cgens:
                    for _ in g:
                        pass

            if dbg:
                nc.sync.dma_start(dbg_q[:], qrot[0][:])
                nc.sync.dma_start(dbg_k[:], krot[0][:])
                nc.sync.dma_start(dbg_v[:], v_sb[:])
                nc.sync.dma_start(dbg_y[:], yt[:])

    nc.compile()
    return nc


def _get_nc():
    global _NC
    if _NC is None:
        _NC = build_nc()
    return _NC


def _prep_inputs(x, w_attn, w_proj):
    """Build the 8 per-core input maps from the full-problem arrays."""
    perm = np.concatenate([np.arange(0, HD, 2), np.arange(1, HD, 2)])

    f = np.arange(64, dtype=np.float64)
    inv = ROPE_THETA ** (-2.0 * f / HD)
    ang = inv[:, None] * np.arange(T, dtype=np.float64)[None, :]
    trigc = np.cos(ang).astype(np.float32)
    trigs = np.sin(ang).astype(np.float32)
    trigf = np.ascontiguousarray(np.concatenate([trigc, trigc], axis=0))
    trigw = np.ascontiguousarray(np.concatenate([trigs, trigs], axis=0))

    kk = np.arange(128)[None, :, None]
    qq = np.arange(512)[None, None, :]
    dd = np.arange(4)[:, None, None]
    maskd = ((128 * dd + kk) <= qq).astype(BF)

    w_attn = np.asarray(w_attn)
    w_proj = np.asarray(w_proj)
    x = np.asarray(x)

    in_maps = []
    for core in range(N_CORES):
        b, g = core // TP, core % TP
        xTa = np.ascontiguousarray(x[b].T).astype(BF)

        qrows = []
        for h in range(HQ):
            gh = g * HQ + h
            qrows.append(gh * HD + perm)
        for kv in range(HKV):
            gk = g * HKV + kv
            qrows.append(N_HEAD * HD + gk * HD + perm)
        qrows = np.concatenate(qrows)
        wqk = w_attn[qrows].astype(BF)  # [1280, C]
        # wqk3[m, p, kc*128+col] = wqk[m*128+col, kc*128+p]
        wqk3 = np.ascontiguousarray(
            wqk.reshape(MQK, 128, KC, 128).transpose(0, 3, 2, 1).reshape(MQK, 128, C)
        )

        vrows = np.concatenate(
            [
                (N_HEAD + N_KV_HEAD) * HD + (g * HKV + kv) * HD + np.arange(HD)
                for kv in range(HKV)
            ]
        )
        wv = w_attn[vrows].astype(BF)  # [256, C]
        # wv3[p, kc*256+c] = wv[c, kc*128+p]
        wv3 = np.ascontiguousarray(
            wv.reshape(HKV * HD, KC, 128).transpose(2, 1, 0).reshape(128, KC * HKV * HD)
        )

        cols = np.arange(g * HQ * HD, (g + 1) * HQ * HD)
        wpg = w_proj[:, cols].astype(BF)  # [C, 1024], rows = out features
        # wp5[fm, d, h, p] = wpg[fm*128+p, h*128+d]
        wp5 = np.ascontiguousarray(
            wpg.T.reshape(HQ, 128, FM, 128).transpose(2, 1, 0, 3)
        )

        in_maps.append(
            {
                "xT": xTa,
                "wqk3": wqk3,
                "wv3": wv3,
                "wp5": wp5,
                "trigf": trigf,
                "trigw": trigw,
                "maskd": maskd,
            }
        )
    return in_maps


def kernel(x, w_attn, w_proj):
    global LAST_RUN
    nc = _get_nc()
    in_maps = _prep_inputs(x, w_attn, w_proj)
    res = run_bass_kernel_spmd(nc, in_maps, core_ids=list(range(N_CORES)))
    LAST_RUN = res
    out = np.empty((B, T, C), dtype=np.float32)
    for b in range(B):
        acc = res.results[TP * b]["outT"] + res.results[TP * b + 1]["outT"]
        out[b] = acc.T
    return out



# revision 28
# speedup vs baseline: 1.1171x; 1.1171x over previous
"""Causal self-attention (GQA + RoPE) Trainium2 kernel, 8-way sharded.

Sharding: DP=4 over batch x TP=2 over kv-head groups (2 kv heads + their
8 q heads per group).  Each core computes its batch's qkv projection for
its head group, causal attention, and a partial c_proj (columns of
w_proj for its head group).  Host sums the two partial c_proj outputs
per batch.

Everything on-chip runs transposed ([feature, token] layout) so matmuls
contract along partitions; host transposes inputs/outputs.

Linear projections (qkv and c_proj) run as fp8e4 DoubleRow matmuls with
a 3-term hi/lo compensation: each operand X is split into
X = X_hi + X_lo (both fp8e4), and the product contributes
W_hi.X_hi + W_lo.X_hi + W_hi.X_lo (the lo.lo term is ~2^-9 relative and
dropped).  DoubleRow packs two contraction tiles per pass, so the three
terms over a pair of 128-deep k-tiles cost 3 passes instead of 4
bf16-equivalents: 0.75x the math at ~bf16 accuracy.  Weights are
pre-scaled by 32 on the host so their lo planes stay in fp8-normal
range; the scale is unwound in the softmax exp scale (1/32^2 on q.k),
the reciprocal broadcast (yt = 8*y_true), and the final c_proj copy
(2^-8).  x is split on the host; yt is split on-chip (Pool casts the hi
plane, DVE subtracts the lo plane).

Attention (QK^T, AV, ones-matmul denominators) stays bf16: its
contraction is a single 128-tile, so DoubleRow pairing has nothing to
pair with and compensation costs more than it saves.

Pipeline: the attention inner loop is ACT-bound (one exp per QK tile),
so the q/k projection + RoPE work for head h+1 is interleaved into the
PE stream of head h's attention, keeping the PE busy while ACT churns
through exps.

RoPE: w_attn q/k rows are permuted per-head to [even dims; odd dims] so
rotation pairs land at partition f and f+64 of the qkv psum tile:
  P  = ps * [c; c] (SBUF),  P2 = ps * [s; s] (PSUM)
  out[0:64]   = P[0:64]  - P2[64:128]
  out[64:128] = P2[0:64] + P[64:128]
(each combine reads one SBUF + one PSUM operand, which may sit at
different base partitions; two SBUF operands may not).

Softmax: att^T tiles ([k, q] layout) are exp'd on ACT without
max-subtraction (logits are O(6), fp32-safe).  Denominators: groups of
4 e-tiles are tree-summed on DVE and hit with one ones-column matmul
per group (deferred into the next group's PE stream); the per-q
reciprocal is broadcast down partitions with a f32r outer-product
matmul, also deferred one q-tile.
"""

import math

import numpy as np
import ml_dtypes

import concourse.bass as bass
import concourse.mybir as mybir
import concourse.tile as tile
from concourse import bacc
from concourse.bass_utils import run_bass_kernel_spmd

ALU = mybir.AluOpType
AF = mybir.ActivationFunctionType
F32 = mybir.dt.float32
F32R = mybir.dt.float32r
BF16 = mybir.dt.bfloat16
FP8 = mybir.dt.float8e4
DRM = mybir.MatmulPerfMode.DoubleRow
BF = ml_dtypes.bfloat16
E4 = ml_dtypes.float8_e4m3

# problem shape (hardcoded per contest rules)
B, T, C = 4, 2048, 2048
N_HEAD, N_KV_HEAD, HD = 16, 4, 128
ROPE_THETA = 10000.0

TP = 2            # head-group shards
DP = 4            # batch shards
HQ = N_HEAD // TP         # 8 q heads per core
HKV = N_KV_HEAD // TP     # 2 kv heads per core
NREP = N_HEAD // N_KV_HEAD  # 4
QK_ROWS = (HQ + HKV) * HD   # 1280
KC = C // 128     # 16 contraction tiles
NQ = T // 512     # 4 token strips
MQK = QK_ROWS // 128  # 10 feature tiles (8 q heads + 2 kv heads)
FM = C // 128     # 16 output feature tiles

WS = 32.0         # host-side weight scale (keeps fp8 lo planes normal)
YS = 8.0          # yt scale (keeps yt fp8 planes normal)
SCALE = 1.0 / (math.sqrt(HD) * WS * WS)   # exp scale: undo q,k weight scale
RECB = YS / WS    # reciprocal-broadcast constant: yt = YS * y_true
OBS = 1.0 / (WS * YS)   # c_proj output copy scale: undo WS * (WS*YS)/WS... =2^-8

N_CORES = 8

_NC = None        # cached compiled Bass module
LAST_RUN = None   # BassKernelResults of the most recent kernel() call


def build_nc(dbg=False):
    nc = bacc.Bacc(None, target_bir_lowering=False, debug=False)

    xhi = nc.declare_dram_parameter("xhi", [C, T], FP8, isOutput=False)
    xlo = nc.declare_dram_parameter("xlo", [C, T], FP8, isOutput=False)
    wqk3h = nc.declare_dram_parameter("wqk3h", [MQK, 128, C], FP8, isOutput=False)
    wqk3l = nc.declare_dram_parameter("wqk3l", [MQK, 128, C], FP8, isOutput=False)
    wv3h = nc.declare_dram_parameter("wv3h", [128, KC * HKV * HD], FP8, isOutput=False)
    wv3l = nc.declare_dram_parameter("wv3l", [128, KC * HKV * HD], FP8, isOutput=False)
    # c_proj weights packed per fm-PAIR: [fp, p, j(fm-in-pair), s(hi/lo), h, d]
    wp5x = nc.declare_dram_parameter(
        "wp5x", [FM // 2, 128, 2 * 2 * HQ * 128], FP8, isOutput=False
    )
    trigf = nc.declare_dram_parameter("trigf", [128, T], F32, isOutput=False)  # [c;c]
    trigw = nc.declare_dram_parameter("trigw", [128, T], F32, isOutput=False)  # [s;s]
    maskd = nc.declare_dram_parameter("maskd", [4, 128, 512], BF16, isOutput=False)
    outT = nc.declare_dram_parameter("outT", [C, T], BF16, isOutput=True)

    with tile.TileContext(nc) as tc:
        with (
            tc.tile_pool(name="const", bufs=1) as const,
            tc.tile_pool(name="persist", bufs=1) as persist,
            tc.tile_pool(name="eb", bufs=8) as eb,
            tc.tile_pool(name="gag", bufs=2) as gag,
            tc.tile_pool(name="rb", bufs=1) as rb,
            tc.tile_pool(name="psS", bufs=2, space="PSUM") as psS,
            tc.tile_pool(name="psY", bufs=2, space="PSUM") as psY,
            tc.tile_pool(name="psD", bufs=2, space="PSUM") as psD,
        ):
            trigf_sb = const.tile([128, T], F32, name="trigf")
            trigw_sb = const.tile([128, T], F32, name="trigw")
            mask_sb = const.tile([128, 4, 512], BF16, name="mask")
            ones_col = const.tile([128, 1], BF16, name="onec")
            ones_row_f = const.tile([1, 128], F32, name="onerf")
            ones_row = const.tile([1, 128], F32R, name="oner")

            qrot = [persist.tile([128, T], BF16, name=f"qrot{h}") for h in range(HQ)]
            krot = [persist.tile([128, T], BF16, name=f"krot{h}") for h in range(HKV)]
            v_sb = persist.tile([128, T // 128, HKV * HD], BF16, name="vtok")
            yth = persist.tile([128, HQ, T], FP8, name="yth")
            ytl = persist.tile([128, HQ, T], FP8, name="ytl")

            state = {"pending": None, "pending_ones": None}

            def finalize(h, qj, ps_y, ps_d):
                qsl = bass.ts(qj, 512)
                rec_f = rb.tile([1, 512], F32, name="recf")
                rec_r = rb.tile([1, 512], F32R, name="recr")
                r_sb = rb.tile([128, 512], F32, name="r")
                t_sb = rb.tile([128, 512], BF16, name="t")
                nc.vector.reciprocal(rec_f[:], ps_d[:])
                with nc.allow_low_precision("f32r recip broadcast"):
                    nc.vector.tensor_copy(rec_r[:], rec_f[:])
                ps_r = psS.tile([128, 512], F32, name="pss")
                nc.tensor.matmul(
                    ps_r[:], ones_row[:], rec_r[:], start=True, stop=True
                )
                nc.scalar.copy(r_sb[:], ps_r[:])
                nc.vector.tensor_tensor(t_sb[:], ps_y[:], r_sb[:], ALU.mult)
                with nc.allow_low_precision("yt fp8 hi/lo split"):
                    # head 7 feeds c_proj immediately; Pool's q7 launch
                    # latency would stall the first c_stream matmuls
                    if h == HQ - 1:
                        nc.scalar.copy(yth[:, h, qsl], t_sb[:])
                    else:
                        nc.gpsimd.tensor_copy(yth[:, h, qsl], t_sb[:])
                    nc.vector.tensor_tensor(
                        ytl[:, h, qsl], t_sb[:], yth[:, h, qsl], ALU.subtract
                    )

            def flush_ones():
                if state["pending_ones"] is not None:
                    po, st, sp, pd = state["pending_ones"]
                    nc.tensor.matmul(pd[:], ones_col[:], po[:], start=st, stop=sp)
                    state["pending_ones"] = None

            def emit_qj(h, qj, pop):
                """Attention for (h, qj): QK tiles, exp, mask, AV, denominators.

                `pop()` is called once per k-tile to interleave filler PE work.
                Returns the (ps_y, ps_d) accumulators (not yet finalized).
                """
                kvh = h // NREP
                qsl = bass.ts(qj, 512)
                ps_y = psY.tile([128, 512], F32, name="psy")
                ps_d = psD.tile([1, 512], F32, name="psd")
                nkt = 4 * qj + 4
                g0 = ga = g2 = None
                for kt in range(nkt):
                    d = kt - 4 * qj
                    # diagonal tile d has valid q-columns only in [128d, 512)
                    lo = 128 * d if d > 0 else 0
                    qlo = qj * 512 + lo
                    ps_s = psS.tile([128, 512], F32, name="pss")
                    nc.tensor.matmul(
                        ps_s[:, lo:512],
                        krot[kvh][:, kt * 128 : (kt + 1) * 128],
                        qrot[h][:, qlo : (qj + 1) * 512],
                        start=True,
                        stop=True,
                    )
                    e = eb.tile([128, 512], BF16, name="e")
                    nc.scalar.activation(
                        e[:, lo:512], ps_s[:, lo:512], AF.Exp, scale=SCALE
                    )
                    if d >= 0:
                        nc.vector.tensor_tensor(
                            e[:, lo:512], e[:, lo:512],
                            mask_sb[:, d, lo:512], ALU.mult,
                        )
                    nc.tensor.matmul(
                        ps_y[:, lo:512],
                        v_sb[:, kt, kvh * HD : (kvh + 1) * HD],
                        e[:, lo:512],
                        start=(kt == 0),
                        stop=(kt == nkt - 1),
                    )
                    if d >= 0:
                        # diagonal group: narrow per-tile ones-matmuls
                        if d == 0:
                            flush_ones()
                        nc.tensor.matmul(
                            ps_d[:, lo:512],
                            ones_col[:],
                            e[:, lo:512],
                            start=(qj == 0 and kt == 0),
                            stop=(kt == nkt - 1),
                        )
                    else:
                        # full groups: tree-sum 4 e-tiles on DVE, one deferred
                        # ones-matmul per group (emitted in a later PE slot so
                        # the PE never waits on the DVE adds).
                        ph = kt % 4
                        if ph == 0:
                            g0 = e
                        elif ph == 1:
                            ga = gag.tile([128, 512], BF16, name="ga")
                            nc.vector.tensor_tensor(ga[:], g0[:], e[:], ALU.add)
                        elif ph == 2:
                            g2 = e
                        else:
                            gs = gag.tile([128, 512], BF16, name="gs")
                            nc.vector.tensor_tensor(gs[:], g2[:], e[:], ALU.add)
                            nc.vector.tensor_tensor(gs[:], gs[:], ga[:], ALU.add)
                            flush_ones()
                            grp = kt // 4
                            state["pending_ones"] = (gs, grp == 0, False, ps_d)
                    pop(kt)
                return ps_y, ps_d

            # ======== projection machinery (lives through heads 0..6) ========
            with (
                tc.tile_pool(name="xa", bufs=1) as xa,
                tc.tile_pool(name="wm", bufs=3) as wm,
                tc.tile_pool(name="ta", bufs=1) as ta,
                tc.tile_pool(name="psA", bufs=1, space="PSUM") as psA,
                tc.tile_pool(name="psP2", bufs=1, space="PSUM") as psP2,
            ):
                xs_h = xa.tile([128, KC, T], FP8, name="xsh")
                xs_l = xa.tile([128, KC, T], FP8, name="xsl")

                def load_wm(m):
                    wh = wm.tile([128, KC, 128], FP8, name="wmh")
                    wl = wm.tile([128, KC, 128], FP8, name="wml")
                    for w, src in ((wh, wqk3h), (wl, wqk3l)):
                        wsrc = src[m, :, :].rearrange("p (kc c) -> p kc c", kc=KC)
                        nc.sync.dma_start(w[:], wsrc[:])
                    return wh, wl

                def qkv_mms(ps, wh, wl, nsl, start_grp=True, stop_grp=True):
                    """3-term DoubleRow fp8 matmuls: full C contraction.

                    Term-major order: the hi.hi sweep only waits on the hi
                    planes, so compute starts before the lo DMAs land.
                    """
                    np_ = KC // 2
                    terms = [(wh, xs_h), (wl, xs_h), (wh, xs_l)]
                    for ti, (w, xs) in enumerate(terms):
                        for kcp in range(np_):
                            kc = 2 * kcp
                            first = start_grp and ti == 0 and kcp == 0
                            last = stop_grp and ti == 2 and kcp == np_ - 1
                            nc.tensor.matmul(
                                ps, w[:, kc : kc + 2, :], xs[:, kc : kc + 2, nsl],
                                start=first, stop=last, perf_mode=DRM,
                            )
                            if kcp % 2 == 1:
                                yield

                def rope_ops(m, n, ps):
                    """The four RoPE ops for one (feature tile, strip) pair."""
                    dst = qrot[m] if m < HQ else krot[m - HQ]
                    nsl = bass.ts(n, 512)
                    pt = ta.tile([128, 512], F32, name="pt")
                    p2 = psP2.tile([128, 512], F32, name="p2")
                    yield nc.vector.tensor_tensor(
                        pt[:], ps[:], trigf_sb[:, nsl], ALU.mult
                    )
                    yield nc.vector.tensor_tensor(
                        p2[:], ps[:], trigw_sb[:, nsl], ALU.mult
                    )
                    yield nc.vector.tensor_tensor(
                        dst[0:64, nsl], pt[0:64, :], p2[64:128, :], ALU.subtract
                    )
                    yield nc.vector.tensor_tensor(
                        dst[64:128, nsl], p2[0:64, :], pt[64:128, :], ALU.add
                    )

                def a_stream(m, pool):
                    wh, wl = load_wm(m)
                    yield
                    for n in range(NQ):
                        nsl = bass.ts(n, 512)
                        ps = pool.tile([128, 512], F32, name="psA")
                        for _ in qkv_mms(ps[:], wh, wl, nsl):
                            yield
                        for _ in rope_ops(m, n, ps):
                            yield

                def load_x_strip(n, split_hi=False):
                    nsl = bass.ts(n, 512)
                    xhi_r = xhi.rearrange("(kc p) t -> p kc t", p=128)
                    xlo_r = xlo.rearrange("(kc p) t -> p kc t", p=128)
                    if split_hi:
                        # quarter-granular hi chunks so the first v-proj
                        # matmuls start after ~1/4 of the strip lands
                        for i in range(4):
                            csl = slice(n * 512 + 128 * i, n * 512 + 128 * (i + 1))
                            nc.sync.dma_start(xs_h[:, :, csl], xhi_r[:, :, csl])
                    else:
                        nc.sync.dma_start(xs_h[:, :, nsl], xhi_r[:, :, nsl])
                    nc.sync.dma_start(xs_l[:, :, nsl], xlo_r[:, :, nsl])

                # ---- A0: v projection + k heads + q head 0 (pure PE phase) ----
                with tc.tile_pool(name="wvp", bufs=1) as wvp:
                    wv_h = wvp.tile([128, KC, HKV * HD], FP8, name="wvh")
                    wv_l = wvp.tile([128, KC, HKV * HD], FP8, name="wvl")
                    nc.sync.dma_start(
                        wv_h[:], wv3h.rearrange("p (kc c) -> p kc c", kc=KC)
                    )
                    xhi_r0 = xhi.rearrange("(kc p) t -> p kc t", p=128)
                    nc.sync.dma_start(xs_h[:, :, 0:128], xhi_r0[:, :, 0:128])
                    nc.sync.dma_start(
                        wv_l[:], wv3l.rearrange("p (kc c) -> p kc c", kc=KC)
                    )
                    for i in range(1, 4):
                        csl = slice(128 * i, 128 * (i + 1))
                        nc.sync.dma_start(xs_h[:, :, csl], xhi_r0[:, :, csl])
                    nc.sync.dma_start(
                        xs_l[:, :, 0:512],
                        xlo.rearrange("(kc p) t -> p kc t", p=128)[:, :, 0:512],
                    )
                    nc.sync.dma_start(trigf_sb[:], trigf[:])
                    nc.sync.dma_start(trigw_sb[:], trigw[:])
                    nc.sync.dma_start(mask_sb[:], maskd.rearrange("d p q -> p d q"))
                    nc.vector.memset(ones_col[:], 1.0)
                    nc.vector.memset(ones_row_f[:], RECB)
                    with nc.allow_low_precision("f32r ones for recip broadcast"):
                        nc.vector.tensor_copy(ones_row[:], ones_row_f[:])
                    wk0 = load_wm(HQ)
                    wk1 = load_wm(HQ + 1)
                    wq0 = load_wm(0)
                    for n in range(NQ):
                        nsl = bass.ts(n, 512)
                        if n + 1 < NQ:
                            load_x_strip(n + 1)
                        for tt in range(4 * n, 4 * n + 4):
                            # reuse the attention-phase psum slots during A0
                            psv = psS.tile([128, 512], F32, name="pss")[
                                :, : HKV * HD
                            ]
                            tsl = slice(tt * 128, (tt + 1) * 128)
                            # x_lo last: its strip DMA is the latest arrival
                            vterms = [(xs_h, wv_h), (xs_h, wv_l), (xs_l, wv_h)]
                            for ti, (xs, w) in enumerate(vterms):
                                for kcp in range(KC // 2):
                                    kc = 2 * kcp
                                    nc.tensor.matmul(
                                        psv[:],
                                        xs[:, kc : kc + 2, tsl],
                                        w[:, kc : kc + 2, :],
                                        start=(ti == 0 and kcp == 0),
                                        stop=(ti == 2 and kcp == KC // 2 - 1),
                                        perf_mode=DRM,
                                    )
                            nc.scalar.copy(v_sb[:, tt, :], psv[:])
                        # rotate 3 psum slots (psA is free during A0) so the
                        # third m-tile never waits on the first one's rope
                        for (m, (wh, wl)), pool in zip(
                            ((HQ, wk0), (HQ + 1, wk1), (0, wq0)), (psA, psY, psY)
                        ):
                            nm = "psA" if pool is psA else "psy"
                            ps = pool.tile([128, 512], F32, name=nm)
                            for _ in qkv_mms(ps[:], wh, wl, nsl):
                                pass
                            for _ in rope_ops(m, n, ps):
                                pass

                # ---- heads 0..6: attention + next head's projection ----
                for h in range(HQ - 1):
                    agen = a_stream(h + 1, psA)

                    def pop(kt, agen=agen):
                        next(agen, None)
                        if kt < 5 or kt >= 10:
                            next(agen, None)

                    for qj in range(NQ):
                        ps_y, ps_d = emit_qj(h, qj, pop)
                        if state["pending"] is not None:
                            finalize(*state["pending"])
                        state["pending"] = (h, qj, ps_y, ps_d)
                    for _ in agen:
                        pass

            # ---- head 7: attention + output projection interleaved ----
            with (
                tc.tile_pool(name="wpc", bufs=1) as wpc,
                tc.tile_pool(name="obp", bufs=3) as obp,
                tc.tile_pool(name="psO", bufs=2, space="PSUM") as psO,
            ):
                # preload ALL c_proj weights (xa pool just freed 8MB); the
                # drain then runs PE-bound with only small out-DMAs
                wpall = wpc.tile([128, FM // 2, 2, 2, HQ, 128], FP8, name="wpall")
                wp_r = wp5x.rearrange(
                    "fp p (j s h d) -> p fp j s h d", j=2, s=2, h=HQ
                )
                nc.sync.dma_start(wpall[:, 0:4], wp_r[:, 0:4])
                nc.sync.dma_start(wpall[:, 4:8], wp_r[:, 4:8])

                def c_stream(n):
                    """Output projection for token strip n (8 fm-pair tiles)."""
                    nsl = bass.ts(n, 512)
                    for fp in range(FM // 2):
                        yield
                        # the very last fm-pair writes out per-half so the
                        # final DMA exposure after the last matmul is small
                        split_out = n == NQ - 1 and fp == FM // 2 - 1
                        ob = obp.tile([128, 2, 512], BF16, name="ob")
                        for j in range(2):
                            ps_o = psO.tile([128, 512], F32, name="pso")
                            terms = [(0, yth), (1, yth), (0, ytl)]
                            for ti, (s, yt) in enumerate(terms):
                                for hp in range(HQ // 2):
                                    h2 = 2 * hp
                                    nc.tensor.matmul(
                                        ps_o[:],
                                        wpall[:, fp, j, s, h2 : h2 + 2, :],
                                        yt[:, h2 : h2 + 2, nsl],
                                        start=(ti == 0 and hp == 0),
                                        stop=(ti == 2 and hp == HQ // 2 - 1),
                                        perf_mode=DRM,
                                    )
                                yield
                            nc.scalar.activation(
                                ob[:, j, :], ps_o[:], AF.Copy, scale=OBS
                            )
                            if split_out:
                                nc.sync.dma_start(
                                    outT[
                                        fp * 256 + j * 128 : fp * 256 + (j + 1) * 128,
                                        nsl,
                                    ],
                                    ob[:, j, :],
                                )
                        if not split_out:
                            nc.sync.dma_start(
                                outT[fp * 256 : (fp + 1) * 256, nsl].rearrange(
                                    "(j p) q -> p j q", j=2
                                ),
                                ob[:],
                            )
                        yield

                cgens = []

                _end = object()

                def pop7(kt):
                    for _ in range(2):
                        while cgens:
                            if next(cgens[0], _end) is _end:
                                cgens.pop(0)
                                continue
                            break

                for qj in range(NQ):
                    ps_y, ps_d = emit_qj(HQ - 1, qj, pop7)
                    flush_ones()
                    if state["pending"] is not None:
                        finalize(*state["pending"])
                        state["pending"] = None
                    finalize(HQ - 1, qj, ps_y, ps_d)
                    cgens.append(c_stream(qj))
                # drain remaining output projection
                for g in cgens:
                    for _ in g:
                        pass

    nc.compile()
    return nc


def _get_nc():
    global _NC
    if _NC is None:
        _NC = build_nc()
    return _NC


def _split_fp8(a):
    """Split float array into (hi, lo) fp8e4 planes with hi + lo ~= a."""
    hi = a.astype(E4)
    lo = (a - hi.astype(a.dtype)).astype(E4)
    return hi, lo


def _prep_inputs(x, w_attn, w_proj):
    """Build the 8 per-core input maps from the full-problem arrays."""
    perm = np.concatenate([np.arange(0, HD, 2), np.arange(1, HD, 2)])

    f = np.arange(64, dtype=np.float64)
    inv = ROPE_THETA ** (-2.0 * f / HD)
    ang = inv[:, None] * np.arange(T, dtype=np.float64)[None, :]
    trigc = np.cos(ang).astype(np.float32)
    trigs = np.sin(ang).astype(np.float32)
    trigf = np.ascontiguousarray(np.concatenate([trigc, trigc], axis=0))
    trigw = np.ascontiguousarray(np.concatenate([trigs, trigs], axis=0))

    kk = np.arange(128)[None, :, None]
    qq = np.arange(512)[None, None, :]
    dd = np.arange(4)[:, None, None]
    maskd = ((128 * dd + kk) <= qq).astype(BF)

    w_attn = np.asarray(w_attn, dtype=np.float32) * np.float32(WS)
    w_proj = np.asarray(w_proj, dtype=np.float32) * np.float32(WS)
    x = np.asarray(x, dtype=np.float32)

    in_maps = []
    for core in range(N_CORES):
        b, g = core // TP, core % TP
        xTa = np.ascontiguousarray(x[b].T)
        xh, xl = _split_fp8(xTa)

        qrows = []
        for h in range(HQ):
            gh = g * HQ + h
            qrows.append(gh * HD + perm)
        for kv in range(HKV):
            gk = g * HKV + kv
            qrows.append(N_HEAD * HD + gk * HD + perm)
        qrows = np.concatenate(qrows)
        wqk = w_attn[qrows]  # [1280, C], scaled
        # wqk3[m, p, kc*128+col] = wqk[m*128+col, kc*128+p]
        wqk3 = np.ascontiguousarray(
            wqk.reshape(MQK, 128, KC, 128).transpose(0, 3, 2, 1).reshape(MQK, 128, C)
        )
        wqk3h, wqk3l = _split_fp8(wqk3)

        vrows = np.concatenate(
            [
                (N_HEAD + N_KV_HEAD) * HD + (g * HKV + kv) * HD + np.arange(HD)
                for kv in range(HKV)
            ]
        )
        wv = w_attn[vrows]  # [256, C], scaled
        # wv3[p, kc*256+c] = wv[c, kc*128+p]
        wv3 = np.ascontiguousarray(
            wv.reshape(HKV * HD, KC, 128).transpose(2, 1, 0).reshape(128, KC * HKV * HD)
        )
        wv3h, wv3l = _split_fp8(wv3)

        cols = np.arange(g * HQ * HD, (g + 1) * HQ * HD)
        wpg = w_proj[:, cols]  # [C, 1024], rows = out features, scaled
        # wp5[fm, d, h, p] = wpg[fm*128+p, h*128+d]
        wp5 = wpg.T.reshape(HQ, 128, FM, 128).transpose(2, 1, 0, 3)
        wp5h, wp5l = _split_fp8(np.ascontiguousarray(wp5))
        # pack per fm-PAIR: wp5x[fp, p, j, s, h, d]
        both = np.stack([wp5h, wp5l], axis=1)  # [FM, s, d(=p of tile), h, 128]
        # both[fm, s, d, h, c]: tile partition dim is d; want [fp, d, j, s, h, c]
        wp5x = np.ascontiguousarray(
            both.reshape(FM // 2, 2, 2, 128, HQ, 128)  # [fp, j, s, d, h, c]
            .transpose(0, 3, 1, 2, 4, 5)               # [fp, d, j, s, h, c]
            .reshape(FM // 2, 128, 2 * 2 * HQ * 128)
        )

        in_maps.append(
            {
                "xhi": xh,
                "xlo": xl,
                "wqk3h": wqk3h,
                "wqk3l": wqk3l,
                "wv3h": wv3h,
                "wv3l": wv3l,
                "wp5x": wp5x,
                "trigf": trigf,
                "trigw": trigw,
                "maskd": maskd,
            }
        )
    return in_maps


def kernel(x, w_attn, w_proj):
    global LAST_RUN
    nc = _get_nc()
    in_maps = _prep_inputs(x, w_attn, w_proj)
    res = run_bass_kernel_spmd(nc, in_maps, core_ids=list(range(N_CORES)))
    LAST_RUN = res
    out = np.empty((B, T, C), dtype=np.float32)
    for b in range(B):
        acc = res.results[TP * b]["outT"].astype(np.float32) + res.results[
            TP * b + 1
        ]["outT"].astype(np.float32)
        out[b] = acc.T
    return out


# revision 43
# speedup vs baseline: 1.2083x; 1.0816x over previous
"""Causal self-attention (GQA + RoPE) Trainium2 kernel, 8-way sharded.

Sharding: DP=4 over batch x TP=2 over kv-head groups (2 kv heads + their
8 q heads per group).  Each core computes its batch's qkv projection for
its head group, causal attention, and a partial c_proj (columns of
w_proj for its head group).  Host sums the two partial c_proj outputs
per batch.

Everything on-chip runs transposed ([feature, token] layout) so matmuls
contract along partitions; host transposes inputs/outputs.

Linear projections (qkv and c_proj) run as fp8e4 DoubleRow matmuls with
a 3-term hi/lo compensation: each operand X is split into
X = X_hi + X_lo (both fp8e4), and the product contributes
W_hi.X_hi + W_lo.X_hi + W_hi.X_lo (the lo.lo term is ~2^-9 relative and
dropped).  DoubleRow packs two contraction tiles per pass, so the three
terms over a pair of 128-deep k-tiles cost 3 passes instead of 4
bf16-equivalents: 0.75x the math at ~bf16 accuracy.  Weights are
pre-scaled by 32 on the host so their lo planes stay in fp8-normal
range; the scale is unwound in the softmax exp scale (1/32^2 on q.k),
the reciprocal broadcast (yt = 8*y_true), and the final c_proj copy
(2^-8).  x is split on the host; yt is split on-chip (Pool casts the hi
plane, DVE subtracts the lo plane).

Attention (QK^T, AV, ones-matmul denominators) stays bf16: its
contraction is a single 128-tile, so DoubleRow pairing has nothing to
pair with and compensation costs more than it saves.

Pipeline: the attention inner loop is ACT-bound (one exp per QK tile),
so the q/k projection + RoPE work for head h+1 is interleaved into the
PE stream of head h's attention, keeping the PE busy while ACT churns
through exps.

RoPE: w_attn q/k rows are permuted per-head to [even dims; odd dims] so
rotation pairs land at partition f and f+64 of the qkv psum tile:
  P  = ps * [c; c] (SBUF),  P2 = ps * [s; s] (PSUM)
  out[0:64]   = P[0:64]  - P2[64:128]
  out[64:128] = P2[0:64] + P[64:128]
(each combine reads one SBUF + one PSUM operand, which may sit at
different base partitions; two SBUF operands may not).

Softmax: att^T tiles ([k, q] layout) are exp'd on ACT without
max-subtraction (logits are O(6), fp32-safe).  Denominators: groups of
4 e-tiles are tree-summed on DVE and hit with one ones-column matmul
per group (deferred into the next group's PE stream); the per-q
reciprocal is broadcast down partitions with a f32r outer-product
matmul, also deferred one q-tile.
"""

import math

import numpy as np
import ml_dtypes

import concourse.bass as bass
import concourse.bass_isa as bass_isa
import concourse.mybir as mybir
import concourse.tile as tile
from concourse import bacc
from concourse.bass_utils import run_bass_kernel_spmd

ALU = mybir.AluOpType
AF = mybir.ActivationFunctionType
F32 = mybir.dt.float32
F32R = mybir.dt.float32r
BF16 = mybir.dt.bfloat16
FP8 = mybir.dt.float8e4
DRM = mybir.MatmulPerfMode.DoubleRow
BF = ml_dtypes.bfloat16
E4 = ml_dtypes.float8_e4m3

# problem shape (hardcoded per contest rules)
B, T, C = 4, 2048, 2048
N_HEAD, N_KV_HEAD, HD = 16, 4, 128
ROPE_THETA = 10000.0

TP = 2            # head-group shards
DP = 4            # batch shards
HQ = N_HEAD // TP         # 8 q heads per core
HKV = N_KV_HEAD // TP     # 2 kv heads per core
NREP = N_HEAD // N_KV_HEAD  # 4
QK_ROWS = (HQ + HKV) * HD   # 1280
KC = C // 128     # 16 contraction tiles
NQ = T // 512     # 4 token strips
MQK = QK_ROWS // 128  # 10 feature tiles (8 q heads + 2 kv heads)
FM = C // 128     # 16 output feature tiles

WS = 32.0         # host-side weight scale (keeps fp8 lo planes normal)
SCALE = 1.0 / (math.sqrt(HD) * WS * WS)   # exp scale: undo q,k weight scale
OBS = 1.0 / (WS * WS)   # c_proj output copy scale: yt carries 32*y_true

N_CORES = 8

_NC = None        # cached compiled Bass module
LAST_RUN = None   # BassKernelResults of the most recent kernel() call


def build_nc(dbg=False):
    nc = bacc.Bacc(None, target_bir_lowering=False, debug=False)

    xhi = nc.declare_dram_parameter("xhi", [C, T], FP8, isOutput=False)
    xlo = nc.declare_dram_parameter("xlo", [C, T], FP8, isOutput=False)
    wqk3h = nc.declare_dram_parameter("wqk3h", [MQK, 128, C], FP8, isOutput=False)
    wqk3l = nc.declare_dram_parameter("wqk3l", [MQK, 128, C], FP8, isOutput=False)
    wv3h = nc.declare_dram_parameter("wv3h", [128, KC * HKV * HD], FP8, isOutput=False)
    wv3l = nc.declare_dram_parameter("wv3l", [128, KC * HKV * HD], FP8, isOutput=False)
    # c_proj weights packed per fm-PAIR: [fp, p, j(fm-in-pair), s(hi/lo), h, d]
    wp5x = nc.declare_dram_parameter(
        "wp5x", [FM // 2, 128, 2 * 2 * HQ * 128], FP8, isOutput=False
    )
    trigf = nc.declare_dram_parameter("trigf", [128, T], BF16, isOutput=False)  # [c;c]
    trigw = nc.declare_dram_parameter("trigw", [128, T], BF16, isOutput=False)  # [s;s]
    maskd = nc.declare_dram_parameter("maskd", [4, 128, 512], BF16, isOutput=False)
    outT = nc.declare_dram_parameter("outT", [C, T], BF16, isOutput=True)

    with tile.TileContext(nc) as tc:
        with (
            tc.tile_pool(name="const", bufs=1) as const,
            tc.tile_pool(name="persist", bufs=1) as persist,
            tc.tile_pool(name="eb", bufs=8) as eb,
            tc.tile_pool(name="gag", bufs=2) as gag,
            tc.tile_pool(name="rb", bufs=1) as rb,
            tc.tile_pool(name="dt", bufs=2) as dtp,
            tc.tile_pool(name="psS", bufs=4, space="PSUM") as psS,
            tc.tile_pool(name="psY", bufs=2, space="PSUM") as psY,
        ):
            trigf_sb = const.tile([128, T], BF16, name="trigf")
            trigw_sb = const.tile([128, T], BF16, name="trigw")
            mask_sb = const.tile([128, 4, 512], BF16, name="mask")

            qrot = [persist.tile([128, T], BF16, name=f"qrot{h}") for h in range(HQ)]
            krot = [persist.tile([128, T], BF16, name=f"krot{h}") for h in range(HKV)]
            v_sb = persist.tile([128, T // 128, HKV * HD], BF16, name="vtok")
            yth = persist.tile([128, HQ, T], FP8, name="yth")
            ytl = persist.tile([128, HQ, T], FP8, name="ytl")

            state = {"pending": None}

            def finalize(h, qj, ps_y, dtot):
                qsl = bass.ts(qj, 512)
                red = rb.tile([128, 512], F32, name="red")
                r_sb = rb.tile([128, 512], F32, name="r")
                t_sb = rb.tile([128, 512], BF16, name="t")
                # GPSIMD all-reduce: every partition of red holds the full
                # denominator, so the reciprocal is already broadcast
                nc.gpsimd.partition_all_reduce(
                    red[:], dtot[:], 128, bass_isa.ReduceOp.add
                )
                nc.vector.reciprocal(r_sb[:], red[:])
                nc.vector.tensor_tensor(t_sb[:], ps_y[:], r_sb[:], ALU.mult)
                with nc.allow_low_precision("yt fp8 hi/lo split"):
                    # head 7 feeds c_proj immediately; Pool's q7 launch
                    # latency would stall the first c_stream matmuls
                    if h == HQ - 1:
                        nc.scalar.copy(yth[:, h, qsl], t_sb[:])
                    else:
                        nc.gpsimd.tensor_copy(yth[:, h, qsl], t_sb[:])
                    nc.vector.tensor_tensor(
                        ytl[:, h, qsl], t_sb[:], yth[:, h, qsl], ALU.subtract
                    )

            def emit_qj(h, qj, pop):
                """Attention for (h, qj): QK tiles, exp, mask, AV, denominators.

                `pop()` is called once per k-tile to interleave filler PE work.
                Returns the (ps_y, dtot) accumulators (not yet finalized).
                Denominators never touch the PE: every aligned group of 4
                e-tiles is tree-summed on DVE (diagonal tiles are exp'd and
                masked at full width so they join a group too), group sums
                accumulate into dtot on Pool, and finalize all-reduces dtot
                across partitions on Pool.
                """
                kvh = h // NREP
                qsl = bass.ts(qj, 512)
                ps_y = psY.tile([128, 512], F32, name="psy")
                dtot = dtp.tile([128, 512], F32, name="dtot")
                nkt = 4 * qj + 4
                g0 = ga = g2 = None
                diag_es = []
                for kt in range(nkt):
                    d = kt - 4 * qj
                    # diagonal tile d has valid q-columns only in [128d, 512)
                    lo = 128 * d if d > 0 else 0
                    qlo = qj * 512 + lo
                    ps_s = psS.tile([128, 512], F32, name="pss")
                    nc.tensor.matmul(
                        ps_s[:, lo:512],
                        krot[kvh][:, kt * 128 : (kt + 1) * 128],
                        qrot[h][:, qlo : (qj + 1) * 512],
                        start=True,
                        stop=True,
                    )
                    e = eb.tile([128, 512], BF16, name="e")
                    nc.scalar.activation(
                        e[:, lo:512], ps_s[:, lo:512], AF.Exp, scale=SCALE
                    )
                    if d >= 0:
                        nc.vector.tensor_tensor(
                            e[:, lo:512], e[:, lo:512],
                            mask_sb[:, d, lo:512], ALU.mult,
                        )
                    nc.tensor.matmul(
                        ps_y[:, lo:512],
                        v_sb[:, kt, kvh * HD : (kvh + 1) * HD],
                        e[:, lo:512],
                        start=(kt == 0),
                        stop=(kt == nkt - 1),
                    )
                    if d >= 0:
                        diag_es.append((lo, e))
                    else:
                        # full groups: tree-sum 4 e-tiles on DVE, accumulate
                        # the group sum into dtot on Pool
                        ph = kt % 4
                        if ph == 0:
                            g0 = e
                        elif ph == 1:
                            ga = gag.tile([128, 512], BF16, name="ga")
                            nc.vector.tensor_tensor(ga[:], g0[:], e[:], ALU.add)
                        elif ph == 2:
                            g2 = e
                        else:
                            gs = gag.tile([128, 512], BF16, name="gs")
                            nc.vector.tensor_tensor(gs[:], g2[:], e[:], ALU.add)
                            nc.vector.tensor_tensor(gs[:], gs[:], ga[:], ALU.add)
                            grp = kt // 4
                            if grp == 0:
                                nc.gpsimd.tensor_copy(dtot[:], gs[:])
                            else:
                                nc.gpsimd.tensor_tensor(
                                    dtot[:], dtot[:], gs[:], ALU.add
                                )
                    pop(kt)
                # deferred diagonal group-sum: d=0 is full width (and masked)
                # so it seeds the sum; later tiles add their valid ranges in
                # place.  Runs after the AVs so it never sits in the DVE FIFO
                # ahead of a mask the PE is waiting on.
                ds = gag.tile([128, 512], BF16, name="gs")
                nc.vector.tensor_copy(ds[:], diag_es[0][1][:])
                for lo, e in diag_es[1:]:
                    nc.vector.tensor_tensor(
                        ds[:, lo:512], ds[:, lo:512], e[:, lo:512], ALU.add
                    )
                if qj == 0:
                    nc.gpsimd.tensor_copy(dtot[:], ds[:])
                else:
                    nc.gpsimd.tensor_tensor(dtot[:], dtot[:], ds[:], ALU.add)
                return ps_y, dtot

            # ======== projection machinery (lives through heads 0..6) ========
            with (
                tc.tile_pool(name="xa", bufs=1) as xa,
                tc.tile_pool(name="wm", bufs=4) as wm,
                tc.tile_pool(name="ta", bufs=1) as ta,
                tc.tile_pool(name="psA", bufs=1, space="PSUM") as psA,
                tc.tile_pool(name="psP2", bufs=1, space="PSUM") as psP2,
            ):
                xs_h = xa.tile([128, KC, T], FP8, name="xsh")
                xs_l = xa.tile([128, KC, T], FP8, name="xsl")

                def load_wm_hi(m):
                    wh = wm.tile([128, KC, 128], FP8, name="wmh")
                    nc.sync.dma_start(
                        wh[:], wqk3h[m, :, :].rearrange("p (kc c) -> p kc c", kc=KC)
                    )
                    return wh

                def load_wm_lo(m):
                    wl = wm.tile([128, KC, 128], FP8, name="wml")
                    nc.sync.dma_start(
                        wl[:], wqk3l[m, :, :].rearrange("p (kc c) -> p kc c", kc=KC)
                    )
                    return wl

                def load_wm(m):
                    return load_wm_hi(m), load_wm_lo(m)

                def qkv_mms(ps, wh, wl, nsl, start_grp=True, stop_grp=True):
                    """3-term DoubleRow fp8 matmuls: full C contraction.

                    Term-major order: the hi.hi sweep only waits on the hi
                    planes, so compute starts before the lo DMAs land.
                    """
                    np_ = KC // 2
                    terms = [(wh, xs_h), (wl, xs_h), (wh, xs_l)]
                    for ti, (w, xs) in enumerate(terms):
                        for kcp in range(np_):
                            kc = 2 * kcp
                            first = start_grp and ti == 0 and kcp == 0
                            last = stop_grp and ti == 2 and kcp == np_ - 1
                            nc.tensor.matmul(
                                ps, w[:, kc : kc + 2, :], xs[:, kc : kc + 2, nsl],
                                start=first, stop=last, perf_mode=DRM,
                            )
                            if kcp % 2 == 1:
                                yield

                def rope_ops(m, n, ps):
                    """The four RoPE ops for one (feature tile, strip) pair."""
                    dst = qrot[m] if m < HQ else krot[m - HQ]
                    nsl = bass.ts(n, 512)
                    pt = ta.tile([128, 512], F32, name="pt")
                    p2 = psP2.tile([128, 512], F32, name="p2")
                    yield nc.vector.tensor_tensor(
                        pt[:], ps[:], trigf_sb[:, nsl], ALU.mult
                    )
                    yield nc.vector.tensor_tensor(
                        p2[:], ps[:], trigw_sb[:, nsl], ALU.mult
                    )
                    yield nc.vector.tensor_tensor(
                        dst[0:64, nsl], pt[0:64, :], p2[64:128, :], ALU.subtract
                    )
                    yield nc.vector.tensor_tensor(
                        dst[64:128, nsl], p2[0:64, :], pt[64:128, :], ALU.add
                    )

                wm_pre = {}

                def a_stream(m, pool):
                    wh, wl = wm_pre.pop(m) if m in wm_pre else load_wm(m)
                    yield
                    for n in range(NQ):
                        nsl = bass.ts(n, 512)
                        ps = pool.tile([128, 512], F32, name="psA")
                        for _ in qkv_mms(ps[:], wh, wl, nsl):
                            yield
                        for _ in rope_ops(m, n, ps):
                            yield

                def load_x_strip(n, split_hi=False):
                    nsl = bass.ts(n, 512)
                    xhi_r = xhi.rearrange("(kc p) t -> p kc t", p=128)
                    xlo_r = xlo.rearrange("(kc p) t -> p kc t", p=128)
                    if split_hi:
                        # quarter-granular hi chunks so the first v-proj
                        # matmuls start after ~1/4 of the strip lands
                        for i in range(4):
                            csl = slice(n * 512 + 128 * i, n * 512 + 128 * (i + 1))
                            nc.sync.dma_start(xs_h[:, :, csl], xhi_r[:, :, csl])
                    else:
                        nc.sync.dma_start(xs_h[:, :, nsl], xhi_r[:, :, nsl])
                    nc.sync.dma_start(xs_l[:, :, nsl], xlo_r[:, :, nsl])

                # ---- A0: v projection + k heads + q head 0 (pure PE phase) ----
                with tc.tile_pool(name="wvp", bufs=1) as wvp:
                    wv_h = wvp.tile([128, KC, HKV * HD], FP8, name="wvh")
                    wv_l = wvp.tile([128, KC, HKV * HD], FP8, name="wvl")
                    nc.sync.dma_start(
                        wv_h[:], wv3h.rearrange("p (kc c) -> p kc c", kc=KC)
                    )
                    xhi_r0 = xhi.rearrange("(kc p) t -> p kc t", p=128)
                    nc.sync.dma_start(xs_h[:, :, 0:128], xhi_r0[:, :, 0:128])
                    nc.sync.dma_start(
                        wv_l[:], wv3l.rearrange("p (kc c) -> p kc c", kc=KC)
                    )
                    for i in range(1, 4):
                        csl = slice(128 * i, 128 * (i + 1))
                        nc.sync.dma_start(xs_h[:, :, csl], xhi_r0[:, :, csl])
                    # hi planes of the three A0 weight tiles land before the
                    # lo-plane x strip: strip 0 has xs_h-only work queued up
                    wk0h = load_wm_hi(HQ)
                    wk1h = load_wm_hi(HQ + 1)
                    wq0h = load_wm_hi(0)
                    nc.sync.dma_start(
                        xs_l[:, :, 0:512],
                        xlo.rearrange("(kc p) t -> p kc t", p=128)[:, :, 0:512],
                    )
                    wk0 = (wk0h, load_wm_lo(HQ))
                    wk1 = (wk1h, load_wm_lo(HQ + 1))
                    wq0 = (wq0h, load_wm_lo(0))
                    load_x_strip(1)
                    nc.sync.dma_start(trigf_sb[:], trigf[:])
                    nc.sync.dma_start(trigw_sb[:], trigw[:])
                    nc.sync.dma_start(mask_sb[:], maskd.rearrange("d p q -> p d q"))

                    a0_m = ((HQ, wk0), (HQ + 1, wk1), (0, wq0))
                    a0_pools = (psA, psY, psY)

                    def mm_sweep(ps, w, xs, nsl, start=False, stop=False):
                        for kcp in range(KC // 2):
                            kc = 2 * kcp
                            nc.tensor.matmul(
                                ps[:], w[:, kc : kc + 2, :], xs[:, kc : kc + 2, nsl],
                                start=(start and kcp == 0),
                                stop=(stop and kcp == KC // 2 - 1),
                                perf_mode=DRM,
                            )

                    for n in range(NQ):
                        nsl = bass.ts(n, 512)
                        if 0 < n < NQ - 1:
                            load_x_strip(n + 1)
                        mps = []
                        if n == 0:
                            # strip 0: open the m-tile groups with their
                            # xs_h-only sweeps first — this is the only PE
                            # work available while xs_l is still in flight
                            for (m, (wh, wl)), pool in zip(a0_m, a0_pools):
                                nm = "psA" if pool is psA else "psy"
                                ps = pool.tile([128, 512], F32, name=nm)
                                mm_sweep(ps, wh, xs_h, nsl, start=True)
                                mm_sweep(ps, wl, xs_h, nsl)
                                mps.append(ps)
                        for tt in range(4 * n, 4 * n + 4):
                            # reuse the attention-phase psum slots during A0
                            psv = psS.tile([128, 512], F32, name="pss")[
                                :, : HKV * HD
                            ]
                            tsl = slice(tt * 128, (tt + 1) * 128)
                            # x_lo last: its strip DMA is the latest arrival
                            vterms = [(xs_h, wv_h), (xs_h, wv_l), (xs_l, wv_h)]
                            for ti, (xs, w) in enumerate(vterms):
                                for kcp in range(KC // 2):
                                    kc = 2 * kcp
                                    nc.tensor.matmul(
                                        psv[:],
                                        xs[:, kc : kc + 2, tsl],
                                        w[:, kc : kc + 2, :],
                                        start=(ti == 0 and kcp == 0),
                                        stop=(ti == 2 and kcp == KC // 2 - 1),
                                        perf_mode=DRM,
                                    )
                            nc.scalar.copy(v_sb[:, tt, :], psv[:])
                        # rotate 3 psum slots (psA is free during A0) so the
                        # third m-tile never waits on the first one's rope
                        for i, ((m, (wh, wl)), pool) in enumerate(zip(a0_m, a0_pools)):
                            if n == 0:
                                ps = mps[i]
                                mm_sweep(ps, wh, xs_l, nsl, stop=True)
                            else:
                                nm = "psA" if pool is psA else "psy"
                                ps = pool.tile([128, 512], F32, name=nm)
                                for _ in qkv_mms(ps[:], wh, wl, nsl):
                                    pass
                            for _ in rope_ops(m, n, ps):
                                pass

                wm_pre[1] = load_wm(1)

                # ---- heads 0..6: attention + next head's projection ----
                for h in range(HQ - 1):
                    if h + 2 <= HQ - 1:
                        # issue head h+2's weight DMAs a full head early so
                        # the next stream's filler is never DMA-gated
                        wm_pre[h + 2] = load_wm(h + 2)
                    agen = a_stream(h + 1, psA)

                    def pop(kt, agen=agen):
                        next(agen, None)
                        if kt < 5 or kt >= 10:
                            next(agen, None)

                    for qj in range(NQ):
                        ps_y, ps_d = emit_qj(h, qj, pop)
                        if state["pending"] is not None:
                            finalize(*state["pending"])
                        state["pending"] = (h, qj, ps_y, ps_d)
                    for _ in agen:
                        pass

            # ---- head 7: attention + output projection interleaved ----
            with (
                tc.tile_pool(name="wpc", bufs=1) as wpc,
                tc.tile_pool(name="obp", bufs=3) as obp,
                tc.tile_pool(name="psO", bufs=2, space="PSUM") as psO,
            ):
                # preload ALL c_proj weights (xa pool just freed 8MB); the
                # drain then runs PE-bound with only small out-DMAs
                wpall = wpc.tile([128, FM // 2, 2, 2, HQ, 128], FP8, name="wpall")
                wp_r = wp5x.rearrange(
                    "fp p (j s h d) -> p fp j s h d", j=2, s=2, h=HQ
                )
                nc.sync.dma_start(wpall[:, 0:4], wp_r[:, 0:4])
                nc.sync.dma_start(wpall[:, 4:8], wp_r[:, 4:8])

                def c_stream(n):
                    """Output projection for token strip n (8 fm-pair tiles)."""
                    nsl = bass.ts(n, 512)
                    for fp in range(FM // 2):
                        yield
                        # the very last fm-pair writes out per-half so the
                        # final DMA exposure after the last matmul is small
                        split_out = n == NQ - 1 and fp == FM // 2 - 1
                        ob = obp.tile([128, 2, 512], BF16, name="ob")
                        for j in range(2):
                            ps_o = psO.tile([128, 512], F32, name="pso")
                            terms = [(0, yth), (1, yth), (0, ytl)]
                            for ti, (s, yt) in enumerate(terms):
                                for hp in range(HQ // 2):
                                    h2 = 2 * hp
                                    nc.tensor.matmul(
                                        ps_o[:],
                                        wpall[:, fp, j, s, h2 : h2 + 2, :],
                                        yt[:, h2 : h2 + 2, nsl],
                                        start=(ti == 0 and hp == 0),
                                        stop=(ti == 2 and hp == HQ // 2 - 1),
                                        perf_mode=DRM,
                                    )
                                yield
                            nc.scalar.activation(
                                ob[:, j, :], ps_o[:], AF.Copy, scale=OBS
                            )
                            if split_out:
                                nc.sync.dma_start(
                                    outT[
                                        fp * 256 + j * 128 : fp * 256 + (j + 1) * 128,
                                        nsl,
                                    ],
                                    ob[:, j, :],
                                )
                        if not split_out:
                            nc.sync.dma_start(
                                outT[fp * 256 : (fp + 1) * 256, nsl].rearrange(
                                    "(j p) q -> p j q", j=2
                                ),
                                ob[:],
                            )
                        yield

                cgens = []

                _end = object()

                def pop7(kt):
                    for _ in range(2):
                        while cgens:
                            if next(cgens[0], _end) is _end:
                                cgens.pop(0)
                                continue
                            break

                for qj in range(NQ):
                    ps_y, ps_d = emit_qj(HQ - 1, qj, pop7)
                    if state["pending"] is not None:
                        finalize(*state["pending"])
                        state["pending"] = None
                    finalize(HQ - 1, qj, ps_y, ps_d)
                    cgens.append(c_stream(qj))
                # drain remaining output projection
                for g in cgens:
                    for _ in g:
                        pass

    nc.compile()
    return nc


def _get_nc():
    global _NC
    if _NC is None:
        _NC = build_nc()
    return _NC


def _split_fp8(a):
    """Split float array into (hi, lo) fp8e4 planes with hi + lo ~= a."""
    hi = a.astype(E4)
    lo = (a - hi.astype(a.dtype)).astype(E4)
    return hi, lo


def _prep_inputs(x, w_attn, w_proj):
    """Build the 8 per-core input maps from the full-problem arrays."""
    perm = np.concatenate([np.arange(0, HD, 2), np.arange(1, HD, 2)])

    f = np.arange(64, dtype=np.float64)
    inv = ROPE_THETA ** (-2.0 * f / HD)
    ang = inv[:, None] * np.arange(T, dtype=np.float64)[None, :]
    trigc = np.cos(ang).astype(np.float32)
    trigs = np.sin(ang).astype(np.float32)
    trigf = np.ascontiguousarray(np.concatenate([trigc, trigc], axis=0)).astype(BF)
    trigw = np.ascontiguousarray(np.concatenate([trigs, trigs], axis=0)).astype(BF)

    kk = np.arange(128)[None, :, None]
    qq = np.arange(512)[None, None, :]
    dd = np.arange(4)[:, None, None]
    maskd = ((128 * dd + kk) <= qq).astype(BF)

    w_attn = np.asarray(w_attn, dtype=np.float32) * np.float32(WS)
    w_proj = np.asarray(w_proj, dtype=np.float32) * np.float32(WS)
    x = np.asarray(x, dtype=np.float32)

    in_maps = []
    for core in range(N_CORES):
        b, g = core // TP, core % TP
        xTa = np.ascontiguousarray(x[b].T)
        xh, xl = _split_fp8(xTa)

        qrows = []
        for h in range(HQ):
            gh = g * HQ + h
            qrows.append(gh * HD + perm)
        for kv in range(HKV):
            gk = g * HKV + kv
            qrows.append(N_HEAD * HD + gk * HD + perm)
        qrows = np.concatenate(qrows)
        wqk = w_attn[qrows]  # [1280, C], scaled
        # wqk3[m, p, kc*128+col] = wqk[m*128+col, kc*128+p]
        wqk3 = np.ascontiguousarray(
            wqk.reshape(MQK, 128, KC, 128).transpose(0, 3, 2, 1).reshape(MQK, 128, C)
        )
        wqk3h, wqk3l = _split_fp8(wqk3)

        vrows = np.concatenate(
            [
                (N_HEAD + N_KV_HEAD) * HD + (g * HKV + kv) * HD + np.arange(HD)
                for kv in range(HKV)
            ]
        )
        wv = w_attn[vrows]  # [256, C], scaled
        # wv3[p, kc*256+c] = wv[c, kc*128+p]
        wv3 = np.ascontiguousarray(
            wv.reshape(HKV * HD, KC, 128).transpose(2, 1, 0).reshape(128, KC * HKV * HD)
        )
        wv3h, wv3l = _split_fp8(wv3)

        cols = np.arange(g * HQ * HD, (g + 1) * HQ * HD)
        wpg = w_proj[:, cols]  # [C, 1024], rows = out features, scaled
        # wp5[fm, d, h, p] = wpg[fm*128+p, h*128+d]
        wp5 = wpg.T.reshape(HQ, 128, FM, 128).transpose(2, 1, 0, 3)
        wp5h, wp5l = _split_fp8(np.ascontiguousarray(wp5))
        # pack per fm-PAIR: wp5x[fp, p, j, s, h, d]
        both = np.stack([wp5h, wp5l], axis=1)  # [FM, s, d(=p of tile), h, 128]
        # both[fm, s, d, h, c]: tile partition dim is d; want [fp, d, j, s, h, c]
        wp5x = np.ascontiguousarray(
            both.reshape(FM // 2, 2, 2, 128, HQ, 128)  # [fp, j, s, d, h, c]
            .transpose(0, 3, 1, 2, 4, 5)               # [fp, d, j, s, h, c]
            .reshape(FM // 2, 128, 2 * 2 * HQ * 128)
        )

        in_maps.append(
            {
                "xhi": xh,
                "xlo": xl,
                "wqk3h": wqk3h,
                "wqk3l": wqk3l,
                "wv3h": wv3h,
                "wv3l": wv3l,
                "wp5x": wp5x,
                "trigf": trigf,
                "trigw": trigw,
                "maskd": maskd,
            }
        )
    return in_maps


def kernel(x, w_attn, w_proj):
    global LAST_RUN
    nc = _get_nc()
    in_maps = _prep_inputs(x, w_attn, w_proj)
    res = run_bass_kernel_spmd(nc, in_maps, core_ids=list(range(N_CORES)))
    LAST_RUN = res
    out = np.empty((B, T, C), dtype=np.float32)
    for b in range(B):
        acc = res.results[TP * b]["outT"].astype(np.float32) + res.results[
            TP * b + 1
        ]["outT"].astype(np.float32)
        out[b] = acc.T
    return out
